# revision 33
# baseline (speedup 1.0000x reference)
"""Trainium2 Bass kernel for nn_NodeNet: GNN message passing + 12-qubit TTN circuit.

Math: the reference's statevector circuit contracts exactly to per-node
Bloch-vector chains (every CNOT block keeps only its target wire; the
measurement is <Z_9>; the circuit is a tree so alive wires stay in
product states). Per node the whole circuit is ~60 scalar ops.

Message passing: Ri/Ro are one-hot column selection matrices, so
  mi = (Ri*e) @ Ro^T @ X = A @ X,   mo = (Ro*e) @ Ri^T @ X = A^T @ X
with A[n,m] = sum_{e: idx_i[e]=n, idx_o[e]=m} e[e] a [1024,1024] graph
matrix built on the host from the weights alone (like the theta->SO(3)
prep). Sharding is then data-parallel over nodes with NO collective:
core k loads A[nk,:]^T and A[:,nk] column-panels (bf16 high+low split,
fp32-grade) and contracts them against the replicated X (also split,
feature-permuted per destination angle slot) in 32 tiny matmuls.

Per-core HBM traffic is ~1.1 MB vs 8 MB for the dense-relation
formulation, and the ReduceScatter (40us wall) is gone entirely.
"""

import ml_dtypes
import numpy as np

import bass_rust
import concourse.bass as bass
import concourse.mybir as mybir
import concourse.tile as tile
from concourse.bass_utils import run_bass_kernel_spmd
from concourse.masks import make_identity

F32 = mybir.dt.float32
F16 = mybir.dt.float16
N_CORES = 8
N, E, D = 1024, 8192, 4
P = 128                  # partitions / nodes per core
NCH = N // P             # 8 global-node chunks (contraction dim)

_BLOCKS = [(0, 1, (0, 1)), (2, 3, (3, 2)), (4, 5, (4, 5)), (6, 7, (7, 6)),
           (8, 9, (8, 9)), (10, 11, (11, 10)), (1, 2, (1, 2)), (5, 6, (6, 5)),
           (9, 10, (10, 9)), (2, 5, (2, 5)), (5, 9, (5, 9))]

# ---------------------------------------------------------------------------
# Column layout of the M-angle tile
# ---------------------------------------------------------------------------
# M cols 0:6  = layer-A target wires  [w1, w6, w10, w2, w5, w9]
# M cols 6:12 = layer-A control wires [w0, w7, w11, w3, w4, w8] (block-paired)
# Sources: wire w<4 -> mi[:,w]; 4<=w<8 -> mo[:,w-4]; w>=8 -> X[:,w-8]
#   mi lands at cols {0,3,6,9} (stride 3): order [mi1, mi2, mi0, mi3]
#   mo lands at cols {1,4,7,10}: order [mo2, mo1, mo3, mo0]
#   X  lands at cols {2,5,8,11}: order [X2, X1, X3, X0]
A_BLOCKS = [0, 3, 5, 1, 2, 4]     # block idx per A-target col
B_BLOCKS = [6, 7, 8]              # b-cols [w2, w5, w9] <- a-cols [w1, w6, w10]
PM_MI = [1, 2, 0, 3]              # mi feature order in M stride-3 slots
PM_MO = [2, 1, 3, 0]              # mo feature order
XK_PERM = [2, 1, 3, 0]            # X columns in M stride-3 order

# ---------------------------------------------------------------------------
# Host-side circuit-constant preparation
# ---------------------------------------------------------------------------

_PAULI = np.array([
    [[0, 1], [1, 0]],
    [[0, -1j], [1j, 0]],
    [[1, 0], [0, -1]],
], dtype=np.complex128)


def _rot_so3(p):
    """SO(3) Bloch rotation of Rot(phi, theta, omega) = RZ(om) RY(th) RZ(phi)."""
    phi, th, om = float(p[0]), float(p[1]), float(p[2])
    c, s = np.cos(th / 2), np.sin(th / 2)
    U = np.array([
        [np.exp(-0.5j * (phi + om)) * c, -np.exp(0.5j * (phi - om)) * s],
        [np.exp(-0.5j * (phi - om)) * s, np.exp(0.5j * (phi + om)) * c],
    ])
    R = np.empty((3, 3))
    for i in range(3):
        for j in range(3):
            R[i, j] = 0.5 * np.real(
                np.trace(_PAULI[i] @ U @ _PAULI[j] @ U.conj().T))
    return R


# circuit-constants column layout (offsets into the ck segment of smalls)
# Layer A groups are 12 wide, col 2t+h (t = A-target col, h = 0:x-term
# j2=0 / 1:z-term j2=2) so one TT against the (q, h) sin view + a
# tensor_reduce over h computes all 6 blocks of one component.
# Layer B groups are 9 wide, col 3s+g (s = B block, g = xyz component).
CK_AZ6 = 0       # layer A control row2: Rc_t[2, j2(h)]
CK_AB = 12       # layer A target: 3 groups of 12, Rt_t[i, j2(h)]
CK_AZ3 = 48      # layer B control row2: Rc_s[2, g]
CK_BB = 57       # layer B target: 3 groups of 9, Rt_s[i, g]
CK_C19 = 84      # R19 column-major: col 3c+i = R19[i, c]
CK_C18 = 93      # R18 row 2 (block 9 control rot)
CK_C21 = 96      # R21 row 2 (block 10 target rot)
CK_C20 = 99      # R20 row 2 (block 10 control rot)
CK_W = 102

# smalls tensor layout: [xk_perm(4) | ck(CK_W)]
SM_XK = 0
SM_CK = 4
SM_W = SM_CK + CK_W


def _pack_ck(theta):
    th = np.asarray(theta, np.float64)
    R = [_rot_so3(th[3 * k:3 * k + 3]) for k in range(23)]
    ck = np.zeros(CK_W, np.float64)

    for t, bidx in enumerate(A_BLOCKS):
        w1, w2, (c, tt) = _BLOCKS[bidx]
        k1, k2 = 2 * bidx, 2 * bidx + 1
        Rc = R[k1] if c == w1 else R[k2]
        Rt = R[k1] if tt == w1 else R[k2]
        for h, j2 in enumerate((0, 2)):
            ck[CK_AZ6 + 2 * t + h] = Rc[2, j2]
            for i in range(3):
                ck[CK_AB + 12 * i + 2 * t + h] = Rt[i, j2]

    for s, bidx in enumerate(B_BLOCKS):
        w1, w2, (c, tt) = _BLOCKS[bidx]
        k1, k2 = 2 * bidx, 2 * bidx + 1
        Rc = R[k1] if c == w1 else R[k2]
        Rt = R[k1] if tt == w1 else R[k2]
        for g in range(3):
            ck[CK_AZ3 + 3 * s + g] = Rc[2, g]
            for i in range(3):
                ck[CK_BB + 9 * i + 3 * s + g] = Rt[i, g]

    # layer C: block 9 = (2,5,(2,5)): control rot R[18] (wire2), target R[19]
    #          block 10 = (5,9,(5,9)): control rot R[20] (wire5), target R[21]
    ck[CK_C19:CK_C19 + 9] = R[19].T.reshape(-1)
    ck[CK_C18:CK_C18 + 3] = R[18][2]
    ck[CK_C21:CK_C21 + 3] = -np.pi * R[21][2]   # folds res = -pi*zf + pi
    ck[CK_C20:CK_C20 + 3] = R[20][2]
    return ck.astype(np.float32)


# ---------------------------------------------------------------------------
# Walrus workaround: this build rejects >1 sync-wait per instruction
# ---------------------------------------------------------------------------


def _split_multi_waits(nc):
    for f in nc.m.functions:
        for bb in f.blocks:
            out = []
            for inst in bb.instructions:
                si = inst.sync_info
                if si is not None and si.on_wait and len(si.on_wait) > 1:
                    waits = list(si.on_wait)
                    for i, w in enumerate(waits[:-1]):
                        out.append(mybir.InstNoOp(
                            name=f"{inst.name}_wsplit{i}",
                            engine=inst.engine,
                            ins=[], outs=[],
                            sync_info=bass_rust.SyncInfo(
                                on_wait=[w], on_update=[]),
                        ))
                    inst.sync_info = bass_rust.SyncInfo(
                        on_wait=[waits[-1]], on_update=list(si.on_update))
                out.append(inst)
            bb.instructions = out


# ---------------------------------------------------------------------------
# Device kernel
# ---------------------------------------------------------------------------


def _build_nc():
    nc = bass.Bass("TRN2", target_bir_lowering=False, num_devices=N_CORES)

    # A-panel layout per rel: chunk c at cols 128c, [128 global, 128 local]
    # fp16 (5e-4 A rel err -> <1e-4 on the output, 200x inside tolerance).
    amat_i = nc.declare_dram_parameter("amat_i", [P, NCH * P], F16,
                                       isOutput=False)
    amat_o = nc.declare_dram_parameter("amat_o", [P, NCH * P], F16,
                                       isOutput=False)
    # X moving: chunk c at cols 8c:8c+8 = [X permMI | X permMO]
    xmov_d = nc.declare_dram_parameter("xmov", [P, NCH * 8], F16,
                                       isOutput=False)
    smalls = nc.declare_dram_parameter("smalls", [P, SM_W], F32,
                                       isOutput=False)
    # single-partition row: a [128,1] output would DMA 128 4-byte
    # partition reads (~7us of descriptor overhead); [1,128] is one burst
    out = nc.declare_dram_parameter("out", [1, P], F32, isOutput=True)

    HPI = float(np.pi / 2)
    PI = float(np.pi)
    MUL = mybir.AluOpType.mult
    ADD = mybir.AluOpType.add

    with tile.TileContext(nc) as tc:
        with (
            tc.tile_pool(name="big", bufs=1) as big,
            tc.tile_pool(name="small", bufs=1) as small,
            tc.tile_pool(name="acc", bufs=2, space="PSUM") as accp,
            tc.tile_pool(name="tbp", bufs=1, space="PSUM") as tbp,
        ):
            # ---- DMAs: A-panel halves prioritized in consumption order ---
            # sync: ap_i0, ap_o0, smalls; scalar: xmov, ap_i1; gpsimd: ap_o1
            ap_sb = {}
            HW = NCH * P // 2      # cols per half-panel
            for rel, src in (("i", amat_i), ("o", amat_o)):
                ap_sb[rel] = [
                    big.tile([P, HW], F16, name=f"ap_{rel}{h}",
                             tag=f"ap_{rel}{h}") for h in range(2)]
            nc.sync.dma_start(ap_sb["i"][0][:], amat_i[:, 0:HW])
            xm_sb = small.tile([P, NCH * 8], F16, name="xm_sb")
            nc.scalar.dma_start(xm_sb[:], xmov_d[:])
            nc.sync.dma_start(ap_sb["o"][0][:], amat_o[:, 0:HW])
            nc.scalar.dma_start(ap_sb["i"][1][:], amat_i[:, HW:2 * HW])
            nc.gpsimd.dma_start(ap_sb["o"][1][:], amat_o[:, HW:2 * HW])
            sm_sb = small.tile([P, SM_W], F32, name="sm_sb")
            nc.sync.dma_start(sm_sb[:], smalls[:])

            def ckc(off, n=1):
                return sm_sb[:, SM_CK + off:SM_CK + off + n]

            # preload the ACT Sin table set while DMAs stream
            warm = small.tile([P, 1], F32, name="warm")
            nc.vector.memset(warm[:], 0.0)
            nc.scalar.activation(warm[:], warm[:],
                                 mybir.ActivationFunctionType.Sin)
            pi_t = small.tile([P, 1], F32, name="pi_t")
            nc.vector.memset(pi_t[:], float(np.pi))
            ident = small.tile([P, P], F32, name="ident")
            make_identity(nc, ident)

            # ---- matmuls: mi = A[nk,:] X, mo = A[:,nk]^T X ---------------
            ps = {}
            for ri, rel in enumerate(("i", "o")):
                ps[rel] = accp.tile([P, 4], F32, name=f"ps_{rel}",
                                    tag=f"ps_{rel}")
            for h in range(2):            # half-panel: chunks 4h..4h+3
                for rel in ("i", "o"):
                    mo_off = 0 if rel == "i" else 4
                    panel = ap_sb[rel][h]
                    for cc in range(NCH // 2):
                        c = 4 * h + cc
                        nc.tensor.matmul(
                            ps[rel][:],
                            panel[:, cc * P:cc * P + P],
                            xm_sb[:, c * 8 + mo_off:c * 8 + mo_off + 4],
                            start=(h == 0 and cc == 0),
                            stop=(h == 1 and cc == 3))

            # ---- circuit: build M angles ---------------------------------
            # cols 0:12 = m (stride-3 interleave), cols 12:24 = m + pi/2
            m_ang = small.tile([P, 24], F32, name="m_ang")
            m3 = m_ang.rearrange("p (c t) -> p c t", t=3)
            nc.vector.tensor_copy(m3[:, 0:4, 0], ps["i"][:])
            nc.vector.tensor_copy(m3[:, 0:4, 1], ps["o"][:])
            nc.gpsimd.tensor_copy(m3[:, 0:4, 2], sm_sb[:, SM_XK:SM_XK + 4])
            nc.vector.tensor_scalar(
                m_ang[:, 12:24], m_ang[:, 0:12], HPI, None, ADD)

            # range-reduce into [-pi, pi] via the magic-constant RNE trick:
            # t = rne(m/2pi) = (m/2pi + 1.5*2^23) - 1.5*2^23; m2 = m - 2pi*t
            TWO_PI = float(2 * np.pi)
            MAGIC = float(1.5 * 2 ** 23)
            tq = small.tile([P, 24], F32, name="tq")
            m2 = small.tile([P, 24], F32, name="m2")
            nc.vector.tensor_scalar(
                tq[:], m_ang[:], float(1.0 / TWO_PI), MAGIC, MUL, ADD)
            nc.vector.tensor_scalar(tq[:], tq[:], -MAGIC, None, ADD)
            nc.vector.scalar_tensor_tensor(
                m2[:], tq[:], -TWO_PI, m_ang[:], MUL, ADD)
            nc.vector.tensor_scalar(
                m2[:], m2[:], PI, -PI,
                mybir.AluOpType.min, mybir.AluOpType.max)
            sxz = small.tile([P, 24], F32, name="sxz")
            nc.scalar.activation(sxz[:], m2[:],
                                 mybir.ActivationFunctionType.Sin)

            TT = nc.vector.tensor_tensor
            TS = nc.vector.tensor_scalar
            STT = nc.vector.scalar_tensor_tensor
            RED = nc.vector.tensor_reduce
            AX = mybir.AxisListType.X

            # sin view indexed (q, h): col = 12h + q; q 0:6 targets (sxb,
            # szb), q 6:12 controls (sxa, sza)
            s_qh = sxz.rearrange("p (h q) -> p q h", h=2)

            # ---- layer A: per component one TT + reduce over h -----------
            # ab_cat = [abx(6) | aby(6) | abz(6)], az6 separate
            ab_cat = small.tile([P, 18], F32, name="ab_cat")
            az6 = small.tile([P, 6], F32, name="az6")
            t12 = small.tile([P, 12], F32, name="t12")
            t12v = t12.rearrange("p (q h) -> p q h", h=2)
            for i in range(3):
                TT(t12[:], ckc(CK_AB + 12 * i, 12), s_qh[:, 0:6, :], MUL)
                RED(ab_cat[:, 6 * i:6 * i + 6], t12v, AX, ADD)
            TT(t12[:], ckc(CK_AZ6, 12), s_qh[:, 6:12, :], MUL)
            RED(az6[:], t12v, AX, ADD)
            TT(ab_cat[:, 6:12], az6[:], ab_cat[:, 6:12], MUL)
            TT(ab_cat[:, 12:18], az6[:], ab_cat[:, 12:18], MUL)

            # ---- layer B: (s, g) views, one TT + reduce per component ----
            # ab_cat viewed (s, g): col = 6g + s; s 0:3 = a-cols, 3:6 b-cols
            ab_sg = ab_cat.rearrange("p (g s) -> p s g", g=3)
            bb_cat = small.tile([P, 9], F32, name="bb_cat")
            az3 = small.tile([P, 3], F32, name="az3")
            t9 = small.tile([P, 9], F32, name="t9")
            t9v = t9.rearrange("p (s g) -> p s g", g=3)
            for i in range(3):
                TT(t9[:], ckc(CK_BB + 9 * i, 9), ab_sg[:, 3:6, :], MUL)
                RED(bb_cat[:, 3 * i:3 * i + 3], t9v, AX, ADD)
            TT(t9[:], ckc(CK_AZ3, 9), ab_sg[:, 0:3, :], MUL)
            RED(az3[:], t9v, AX, ADD)
            TT(bb_cat[:, 3:6], az3[:], bb_cat[:, 3:6], MUL)
            TT(bb_cat[:, 6:9], az3[:], bb_cat[:, 6:9], MUL)

            # ---- layer C: blocks 9 then 10 -------------------------------
            # bb_cat cols: comp c of wire w at 3c + w' (w' 0=w2, 1=w5, 2=w9)
            # STT accum_out fuses each 3-term dot into one instruction;
            # CK_C21 is pre-scaled by -pi on the host so the final result
            # is a single fused multiply-add against the pi constant.
            s9 = small.tile([P, 1], F32, name="s9")
            u = small.tile([P, 1], F32, name="u")
            t3 = small.tile([P, 3], F32, name="t3")
            STT(t3[:], ckc(CK_C18, 3), 1.0, bb_cat[:, 0:9:3], MUL, MUL,
                accum_out=s9[:])
            STT(t3[:], ckc(CK_C21, 3), 1.0, bb_cat[:, 2:9:3], MUL, MUL,
                accum_out=u[:])

            # w5 rows via ck-scalar broadcast: w5cat[i] = sum_c R19[i,c]*bb_c1
            w5c = small.tile([P, 3], F32, name="w5c")
            TS(w5c[:], ckc(CK_C19, 3), bb_cat[:, 1:2], None, MUL)
            STT(w5c[:], ckc(CK_C19 + 3, 3), bb_cat[:, 4:5], w5c[:], MUL, ADD)
            STT(w5c[:], ckc(CK_C19 + 6, 3), bb_cat[:, 7:8], w5c[:], MUL, ADD)
            TS(w5c[:, 1:3], w5c[:, 1:3], s9[:, 0:1], None, MUL)

            s10 = small.tile([P, 1], F32, name="s10")
            STT(t3[:], ckc(CK_C20, 3), 1.0, w5c[:], MUL, MUL,
                accum_out=s10[:])

            # res = s10 * (-pi*u) + pi
            res = small.tile([P, 1], F32, name="res")
            STT(res[:], s10[:], u[:, 0:1], pi_t[:], MUL, ADD)
            row_ps = tbp.tile([1, P], F32, name="row_ps", tag="row")
            nc.tensor.transpose(row_ps[:], res[:], ident[:])
            row_sb = small.tile([1, P], F32, name="row_sb")
            nc.vector.tensor_copy(row_sb[:], row_ps[:])
            nc.scalar.dma_start(out[:], row_sb[:])

    return nc


_NC_CACHE = {}
_RUN_KWARGS = {}      # test harness can set e.g. {"trace": True}
_LAST_RESULTS = []    # BassKernelResults of the most recent run


def _get_nc():
    if "nc" not in _NC_CACHE:
        nc = _build_nc()
        _split_multi_waits(nc)
        _NC_CACHE["nc"] = nc
    return _NC_CACHE["nc"]


def _build_graph_matrix(e, Ri, Ro):
    """A[n,m] = sum over edges (idx_i=n, idx_o=m) of e, in float64."""
    e64 = np.asarray(e, np.float64)
    Ri32 = np.asarray(Ri, np.float32)
    Ro32 = np.asarray(Ro, np.float32)
    idx_i = np.argmax(Ri32, axis=0)
    idx_o = np.argmax(Ro32, axis=0)
    if (np.count_nonzero(Ri32) == E and np.count_nonzero(Ro32) == E
            and np.all(Ri32[idx_i, np.arange(E)] == 1.0)
            and np.all(Ro32[idx_o, np.arange(E)] == 1.0)):
        A = np.zeros((N, N), np.float64)
        np.add.at(A, (idx_i, idx_o), e64)
        return A
    # general fallback (never hit for one-hot relation inputs)
    return (Ri32.astype(np.float64) * e64) @ Ro32.astype(np.float64).T


def _pack_panel(M64):
    """[1024, 128] float64 -> [128, 1024] fp16: chunk c at cols 128c,
    partition p = global row 128c+p."""
    r = M64.astype(np.float16).reshape(NCH, P, P)
    return np.ascontiguousarray(
        r.transpose(1, 0, 2).reshape(P, NCH * P))


def kernel(X, e, Ri, Ro, theta):
    X = np.ascontiguousarray(np.asarray(X, np.float32))
    e = np.ascontiguousarray(np.asarray(e, np.float32))
    theta = np.asarray(theta, np.float32)

    A = _build_graph_matrix(e, Ri, Ro)
    ck1 = _pack_ck(theta)

    # X moving operand: fp16, feature-permuted per destination angle slot
    xm = np.zeros((NCH, P, 8), np.float32)
    xr = X.reshape(NCH, P, D)
    xm[:, :, 0:4] = xr[:, :, PM_MI]
    xm[:, :, 4:8] = xr[:, :, PM_MO]
    xmov = np.ascontiguousarray(
        xm.transpose(1, 0, 2).reshape(P, NCH * 8).astype(np.float16))

    in_maps = []
    for k in range(N_CORES):
        nk = slice(k * P, (k + 1) * P)
        sm = np.empty((P, SM_W), np.float32)
        sm[:, SM_XK:SM_XK + 4] = X[nk][:, XK_PERM]
        sm[:, SM_CK:] = ck1[None, :]
        in_maps.append({
            "amat_i": _pack_panel(np.ascontiguousarray(A[nk, :].T)),
            "amat_o": _pack_panel(np.ascontiguousarray(A[:, nk])),
            "xmov": xmov,
            "smalls": np.ascontiguousarray(sm),
        })

    nc = _get_nc()
    res = run_bass_kernel_spmd(nc, in_maps, core_ids=list(range(N_CORES)),
                               **_RUN_KWARGS)
    _LAST_RESULTS.clear()
    _LAST_RESULTS.append(res)
    return np.concatenate(
        [res.results[k]["out"].reshape(-1) for k in range(N_CORES)]
    ).astype(np.float32)


# revision 39
# speedup vs baseline: 1.0388x; 1.0388x over previous
"""Trainium2 Bass kernel for nn_NodeNet: GNN message passing + 12-qubit TTN circuit.

Math: the reference's statevector circuit contracts exactly to per-node
Bloch-vector chains (every CNOT block keeps only its target wire; the
measurement is <Z_9>; the circuit is a tree so alive wires stay in
product states). Per node the whole circuit is ~60 scalar ops.

Message passing: Ri/Ro are one-hot column selection matrices, so
  mi = (Ri*e) @ Ro^T @ X = A @ X,   mo = (Ro*e) @ Ri^T @ X = A^T @ X
with A[n,m] = sum_{e: idx_i[e]=n, idx_o[e]=m} e[e] a [1024,1024] graph
matrix built on the host from the weights alone (like the theta->SO(3)
prep). Sharding is then data-parallel over nodes with NO collective:
core k loads A[nk,:]^T and A[:,nk] column-panels (bf16 high+low split,
fp32-grade) and contracts them against the replicated X (also split,
feature-permuted per destination angle slot) in 32 tiny matmuls.

Per-core HBM traffic is ~1.1 MB vs 8 MB for the dense-relation
formulation, and the ReduceScatter (40us wall) is gone entirely.
"""

import ml_dtypes
import numpy as np

import bass_rust
import concourse.bass as bass
import concourse.mybir as mybir
import concourse.tile as tile
from concourse.bass_utils import run_bass_kernel_spmd
from concourse.masks import make_identity

F32 = mybir.dt.float32
F16 = mybir.dt.float16
N_CORES = 8
N, E, D = 1024, 8192, 4
P = 128                  # partitions / nodes per core
NCH = N // P             # 8 global-node chunks (contraction dim)

_BLOCKS = [(0, 1, (0, 1)), (2, 3, (3, 2)), (4, 5, (4, 5)), (6, 7, (7, 6)),
           (8, 9, (8, 9)), (10, 11, (11, 10)), (1, 2, (1, 2)), (5, 6, (6, 5)),
           (9, 10, (10, 9)), (2, 5, (2, 5)), (5, 9, (5, 9))]

# ---------------------------------------------------------------------------
# Column layout of the M-angle tile
# ---------------------------------------------------------------------------
# M cols 0:6  = layer-A target wires  [w1, w6, w10, w2, w5, w9]
# M cols 6:12 = layer-A control wires [w0, w7, w11, w3, w4, w8] (block-paired)
# Sources: wire w<4 -> mi[:,w]; 4<=w<8 -> mo[:,w-4]; w>=8 -> X[:,w-8]
#   mi lands at cols {0,3,6,9} (stride 3): order [mi1, mi2, mi0, mi3]
#   mo lands at cols {1,4,7,10}: order [mo2, mo1, mo3, mo0]
#   X  lands at cols {2,5,8,11}: order [X2, X1, X3, X0]
A_BLOCKS = [0, 3, 5, 1, 2, 4]     # block idx per A-target col
B_BLOCKS = [6, 7, 8]              # b-cols [w2, w5, w9] <- a-cols [w1, w6, w10]
PM_MI = [1, 2, 0, 3]              # mi feature order in M stride-3 slots
PM_MO = [2, 1, 3, 0]              # mo feature order
XK_PERM = [2, 1, 3, 0]            # X columns in M stride-3 order

# ---------------------------------------------------------------------------
# Host-side circuit-constant preparation
# ---------------------------------------------------------------------------

_PAULI = np.array([
    [[0, 1], [1, 0]],
    [[0, -1j], [1j, 0]],
    [[1, 0], [0, -1]],
], dtype=np.complex128)


def _rot_so3(p):
    """SO(3) Bloch rotation of Rot(phi, theta, omega) = RZ(om) RY(th) RZ(phi)."""
    phi, th, om = float(p[0]), float(p[1]), float(p[2])
    c, s = np.cos(th / 2), np.sin(th / 2)
    U = np.array([
        [np.exp(-0.5j * (phi + om)) * c, -np.exp(0.5j * (phi - om)) * s],
        [np.exp(-0.5j * (phi - om)) * s, np.exp(0.5j * (phi + om)) * c],
    ])
    R = np.empty((3, 3))
    for i in range(3):
        for j in range(3):
            R[i, j] = 0.5 * np.real(
                np.trace(_PAULI[i] @ U @ _PAULI[j] @ U.conj().T))
    return R


# circuit-constants column layout (offsets into the ck segment of smalls)
# Layer A groups are 12 wide, col 2t+h (t = A-target col, h = 0:x-term
# j2=0 / 1:z-term j2=2) so one TT against the (q, h) sin view + a
# tensor_reduce over h computes all 6 blocks of one component.
# Layer B groups are 9 wide, col 3s+g (s = B block, g = xyz component).
CK_AZ6 = 0       # layer A control row2: Rc_t[2, j2(h)]
CK_AB = 12       # layer A target: 3 groups of 12, Rt_t[i, j2(h)]
CK_AZ3 = 48      # layer B control row2: Rc_s[2, g]
CK_BB = 57       # layer B target: 3 groups of 9, Rt_s[i, g]
CK_C19 = 84      # R19 column-major: col 3c+i = R19[i, c]
CK_C18 = 93      # R18 row 2 (block 9 control rot)
CK_C21 = 96      # R21 row 2 (block 10 target rot)
CK_C20 = 99      # R20 row 2 (block 10 control rot)
CK_W = 102

# smalls tensor layout: [xk_perm(4) | ck(CK_W)]
SM_XK = 0
SM_CK = 4
SM_W = SM_CK + CK_W


def _pack_ck(theta):
    th = np.asarray(theta, np.float64)
    R = [_rot_so3(th[3 * k:3 * k + 3]) for k in range(23)]
    ck = np.zeros(CK_W, np.float64)

    for t, bidx in enumerate(A_BLOCKS):
        w1, w2, (c, tt) = _BLOCKS[bidx]
        k1, k2 = 2 * bidx, 2 * bidx + 1
        Rc = R[k1] if c == w1 else R[k2]
        Rt = R[k1] if tt == w1 else R[k2]
        for h, j2 in enumerate((0, 2)):
            ck[CK_AZ6 + 2 * t + h] = Rc[2, j2]
            for i in range(3):
                ck[CK_AB + 12 * i + 2 * t + h] = Rt[i, j2]

    for s, bidx in enumerate(B_BLOCKS):
        w1, w2, (c, tt) = _BLOCKS[bidx]
        k1, k2 = 2 * bidx, 2 * bidx + 1
        Rc = R[k1] if c == w1 else R[k2]
        Rt = R[k1] if tt == w1 else R[k2]
        for g in range(3):
            ck[CK_AZ3 + 3 * s + g] = Rc[2, g]
            for i in range(3):
                ck[CK_BB + 9 * i + 3 * s + g] = Rt[i, g]

    # layer C: block 9 = (2,5,(2,5)): control rot R[18] (wire2), target R[19]
    #          block 10 = (5,9,(5,9)): control rot R[20] (wire5), target R[21]
    ck[CK_C19:CK_C19 + 9] = R[19].T.reshape(-1)
    ck[CK_C18:CK_C18 + 3] = R[18][2]
    ck[CK_C21:CK_C21 + 3] = -np.pi * R[21][2]   # folds res = -pi*zf + pi
    ck[CK_C20:CK_C20 + 3] = R[20][2]
    return ck.astype(np.float32)


# ---------------------------------------------------------------------------
# Walrus workaround: this build rejects >1 sync-wait per instruction
# ---------------------------------------------------------------------------


def _split_multi_waits(nc):
    for f in nc.m.functions:
        for bb in f.blocks:
            out = []
            for inst in bb.instructions:
                si = inst.sync_info
                if si is not None and si.on_wait and len(si.on_wait) > 1:
                    waits = list(si.on_wait)
                    for i, w in enumerate(waits[:-1]):
                        out.append(mybir.InstNoOp(
                            name=f"{inst.name}_wsplit{i}",
                            engine=inst.engine,
                            ins=[], outs=[],
                            sync_info=bass_rust.SyncInfo(
                                on_wait=[w], on_update=[]),
                        ))
                    inst.sync_info = bass_rust.SyncInfo(
                        on_wait=[waits[-1]], on_update=list(si.on_update))
                out.append(inst)
            bb.instructions = out


# ---------------------------------------------------------------------------
# Device kernel
# ---------------------------------------------------------------------------


def _build_nc():
    nc = bass.Bass("TRN2", target_bir_lowering=False, num_devices=N_CORES)

    # A panels fp16 (5e-4 A rel err -> <1e-4 on the output, 200x inside
    # tolerance), both rels interleaved by half so each DMA piece moves
    # 2 KB per partition row (the efficient DMA line size):
    # cols [0:512 i-chunks 0-3 | 512:1024 o-chunks 0-3 | i 4-7 | o 4-7]
    amat = nc.declare_dram_parameter("amat", [P, 2 * NCH * P], F16,
                                     isOutput=False)
    # X moving: chunk c at cols 8c:8c+8 = [X permMI | X permMO]
    xmov_d = nc.declare_dram_parameter("xmov", [P, NCH * 8], F16,
                                       isOutput=False)
    smalls = nc.declare_dram_parameter("smalls", [P, SM_W], F32,
                                       isOutput=False)
    # single-partition row: a [128,1] output would DMA 128 4-byte
    # partition reads (~7us of descriptor overhead); [1,128] is one burst
    out = nc.declare_dram_parameter("out", [1, P], F32, isOutput=True)

    HPI = float(np.pi / 2)
    PI = float(np.pi)
    MUL = mybir.AluOpType.mult
    ADD = mybir.AluOpType.add

    with tile.TileContext(nc) as tc:
        with (
            tc.tile_pool(name="big", bufs=1) as big,
            tc.tile_pool(name="small", bufs=1) as small,
            tc.tile_pool(name="acc", bufs=2, space="PSUM") as accp,
            tc.tile_pool(name="tbp", bufs=1, space="PSUM") as tbp,
        ):
            # ---- DMAs: two [P, 1024] pieces, 2 KB per partition row ------
            HW = NCH * P // 2      # cols per half-panel
            pieces = [big.tile([P, 2 * HW], F16, name=f"piece{h}",
                               tag=f"piece{h}") for h in range(2)]
            nc.sync.dma_start(pieces[0][:], amat[:, 0:2 * HW])
            xm_sb = small.tile([P, NCH * 8], F16, name="xm_sb")
            nc.scalar.dma_start(xm_sb[:], xmov_d[:])
            nc.gpsimd.dma_start(pieces[1][:], amat[:, 2 * HW:4 * HW])
            sm_sb = small.tile([P, SM_W], F32, name="sm_sb")
            nc.scalar.dma_start(sm_sb[:], smalls[:])

            def ckc(off, n=1):
                return sm_sb[:, SM_CK + off:SM_CK + off + n]

            # preload the ACT Sin table set while DMAs stream
            warm = small.tile([P, 1], F32, name="warm")
            nc.vector.memset(warm[:], 0.0)
            nc.scalar.activation(warm[:], warm[:],
                                 mybir.ActivationFunctionType.Sin)
            pi_t = small.tile([P, 1], F32, name="pi_t")
            nc.vector.memset(pi_t[:], float(np.pi))
            ident = small.tile([P, P], F32, name="ident")
            make_identity(nc, ident)

            # ---- matmuls: mi = A[nk,:] X, mo = A[:,nk]^T X ---------------
            ps = {}
            for ri, rel in enumerate(("i", "o")):
                ps[rel] = accp.tile([P, 4], F32, name=f"ps_{rel}",
                                    tag=f"ps_{rel}")
            for h in range(2):            # half-panel: chunks 4h..4h+3
                for rel in ("i", "o"):
                    mo_off = 0 if rel == "i" else 4
                    rel_off = 0 if rel == "i" else HW
                    for cc in range(NCH // 2):
                        c = 4 * h + cc
                        nc.tensor.matmul(
                            ps[rel][:],
                            pieces[h][:, rel_off + cc * P:
                                      rel_off + cc * P + P],
                            xm_sb[:, c * 8 + mo_off:c * 8 + mo_off + 4],
                            start=(h == 0 and cc == 0),
                            stop=(h == 1 and cc == 3))

            # ---- circuit: build M angles ---------------------------------
            # cols 0:12 = m (stride-3 interleave), cols 12:24 = m + pi/2
            m_ang = small.tile([P, 24], F32, name="m_ang")
            m3 = m_ang.rearrange("p (c t) -> p c t", t=3)
            nc.vector.tensor_copy(m3[:, 0:4, 0], ps["i"][:])
            nc.vector.tensor_copy(m3[:, 0:4, 1], ps["o"][:])
            nc.gpsimd.tensor_copy(m3[:, 0:4, 2], sm_sb[:, SM_XK:SM_XK + 4])
            nc.vector.tensor_scalar(
                m_ang[:, 12:24], m_ang[:, 0:12], HPI, None, ADD)

            # range-reduce into [-pi, pi] via the magic-constant RNE trick:
            # t = rne(m/2pi) = (m/2pi + 1.5*2^23) - 1.5*2^23; m2 = m - 2pi*t
            TWO_PI = float(2 * np.pi)
            MAGIC = float(1.5 * 2 ** 23)
            tq = small.tile([P, 24], F32, name="tq")
            m2 = small.tile([P, 24], F32, name="m2")
            nc.vector.tensor_scalar(
                tq[:], m_ang[:], float(1.0 / TWO_PI), MAGIC, MUL, ADD)
            nc.vector.tensor_scalar(tq[:], tq[:], -MAGIC, None, ADD)
            nc.vector.scalar_tensor_tensor(
                m2[:], tq[:], -TWO_PI, m_ang[:], MUL, ADD)
            nc.vector.tensor_scalar(
                m2[:], m2[:], PI, -PI,
                mybir.AluOpType.min, mybir.AluOpType.max)
            sxz = small.tile([P, 24], F32, name="sxz")
            nc.scalar.activation(sxz[:], m2[:],
                                 mybir.ActivationFunctionType.Sin)

            TT = nc.vector.tensor_tensor
            TS = nc.vector.tensor_scalar
            STT = nc.vector.scalar_tensor_tensor
            RED = nc.vector.tensor_reduce
            AX = mybir.AxisListType.X

            # sin view indexed (q, h): col = 12h + q; q 0:6 targets (sxb,
            # szb), q 6:12 controls (sxa, sza)
            s_qh = sxz.rearrange("p (h q) -> p q h", h=2)

            # ---- layer A: per component one TT + reduce over h -----------
            # ab_cat = [abx(6) | aby(6) | abz(6)], az6 separate
            ab_cat = small.tile([P, 18], F32, name="ab_cat")
            az6 = small.tile([P, 6], F32, name="az6")
            t12 = small.tile([P, 12], F32, name="t12")
            t12v = t12.rearrange("p (q h) -> p q h", h=2)
            for i in range(3):
                TT(t12[:], ckc(CK_AB + 12 * i, 12), s_qh[:, 0:6, :], MUL)
                RED(ab_cat[:, 6 * i:6 * i + 6], t12v, AX, ADD)
            TT(t12[:], ckc(CK_AZ6, 12), s_qh[:, 6:12, :], MUL)
            RED(az6[:], t12v, AX, ADD)
            TT(ab_cat[:, 6:12], az6[:], ab_cat[:, 6:12], MUL)
            TT(ab_cat[:, 12:18], az6[:], ab_cat[:, 12:18], MUL)

            # ---- layer B: (s, g) views, one TT + reduce per component ----
            # ab_cat viewed (s, g): col = 6g + s; s 0:3 = a-cols, 3:6 b-cols
            ab_sg = ab_cat.rearrange("p (g s) -> p s g", g=3)
            bb_cat = small.tile([P, 9], F32, name="bb_cat")
            az3 = small.tile([P, 3], F32, name="az3")
            t9 = small.tile([P, 9], F32, name="t9")
            t9v = t9.rearrange("p (s g) -> p s g", g=3)
            for i in range(3):
                TT(t9[:], ckc(CK_BB + 9 * i, 9), ab_sg[:, 3:6, :], MUL)
                RED(bb_cat[:, 3 * i:3 * i + 3], t9v, AX, ADD)
            TT(t9[:], ckc(CK_AZ3, 9), ab_sg[:, 0:3, :], MUL)
            RED(az3[:], t9v, AX, ADD)
            TT(bb_cat[:, 3:6], az3[:], bb_cat[:, 3:6], MUL)
            TT(bb_cat[:, 6:9], az3[:], bb_cat[:, 6:9], MUL)

            # ---- layer C: blocks 9 then 10 -------------------------------
            # bb_cat cols: comp c of wire w at 3c + w' (w' 0=w2, 1=w5, 2=w9)
            # STT accum_out fuses each 3-term dot into one instruction;
            # CK_C21 is pre-scaled by -pi on the host so the final result
            # is a single fused multiply-add against the pi constant.
            s9 = small.tile([P, 1], F32, name="s9")
            u = small.tile([P, 1], F32, name="u")
            t3 = small.tile([P, 3], F32, name="t3")
            STT(t3[:], ckc(CK_C18, 3), 1.0, bb_cat[:, 0:9:3], MUL, MUL,
                accum_out=s9[:])
            STT(t3[:], ckc(CK_C21, 3), 1.0, bb_cat[:, 2:9:3], MUL, MUL,
                accum_out=u[:])

            # w5 rows via ck-scalar broadcast: w5cat[i] = sum_c R19[i,c]*bb_c1
            w5c = small.tile([P, 3], F32, name="w5c")
            TS(w5c[:], ckc(CK_C19, 3), bb_cat[:, 1:2], None, MUL)
            STT(w5c[:], ckc(CK_C19 + 3, 3), bb_cat[:, 4:5], w5c[:], MUL, ADD)
            STT(w5c[:], ckc(CK_C19 + 6, 3), bb_cat[:, 7:8], w5c[:], MUL, ADD)
            TS(w5c[:, 1:3], w5c[:, 1:3], s9[:, 0:1], None, MUL)

            s10 = small.tile([P, 1], F32, name="s10")
            STT(t3[:], ckc(CK_C20, 3), 1.0, w5c[:], MUL, MUL,
                accum_out=s10[:])

            # res = s10 * (-pi*u) + pi
            res = small.tile([P, 1], F32, name="res")
            STT(res[:], s10[:], u[:, 0:1], pi_t[:], MUL, ADD)
            row_ps = tbp.tile([1, P], F32, name="row_ps", tag="row")
            nc.tensor.transpose(row_ps[:], res[:], ident[:])
            row_sb = small.tile([1, P], F32, name="row_sb")
            nc.vector.tensor_copy(row_sb[:], row_ps[:])
            nc.scalar.dma_start(out[:], row_sb[:])

    return nc


_NC_CACHE = {}
_RUN_KWARGS = {}      # test harness can set e.g. {"trace": True}
_LAST_RESULTS = []    # BassKernelResults of the most recent run


def _get_nc():
    if "nc" not in _NC_CACHE:
        nc = _build_nc()
        _split_multi_waits(nc)
        _NC_CACHE["nc"] = nc
    return _NC_CACHE["nc"]


def _build_graph_matrix(e, Ri, Ro):
    """A[n,m] = sum over edges (idx_i=n, idx_o=m) of e, in float64."""
    e64 = np.asarray(e, np.float64)
    Ri32 = np.asarray(Ri, np.float32)
    Ro32 = np.asarray(Ro, np.float32)
    idx_i = np.argmax(Ri32, axis=0)
    idx_o = np.argmax(Ro32, axis=0)
    if (np.count_nonzero(Ri32) == E and np.count_nonzero(Ro32) == E
            and np.all(Ri32[idx_i, np.arange(E)] == 1.0)
            and np.all(Ro32[idx_o, np.arange(E)] == 1.0)):
        A = np.zeros((N, N), np.float64)
        np.add.at(A, (idx_i, idx_o), e64)
        return A
    # general fallback (never hit for one-hot relation inputs)
    return (Ri32.astype(np.float64) * e64) @ Ro32.astype(np.float64).T


def _pack_amat(Mi64, Mo64):
    """Two [1024, 128] panels -> [128, 2048] fp16: piece h holds chunks
    4h..4h+3 of rel i then rel o; within a rel, chunk c at cols 128c."""
    ri = Mi64.astype(np.float16).reshape(2, 4, P, P)   # [h, cc, p, l]
    ro = Mo64.astype(np.float16).reshape(2, 4, P, P)
    both = np.stack([ri, ro], axis=1)                  # [h, rel, cc, p, l]
    return np.ascontiguousarray(
        both.transpose(3, 0, 1, 2, 4).reshape(P, 2 * NCH * P))


def kernel(X, e, Ri, Ro, theta):
    X = np.ascontiguousarray(np.asarray(X, np.float32))
    e = np.ascontiguousarray(np.asarray(e, np.float32))
    theta = np.asarray(theta, np.float32)

    A = _build_graph_matrix(e, Ri, Ro)
    ck1 = _pack_ck(theta)

    # X moving operand: fp16, feature-permuted per destination angle slot
    xm = np.zeros((NCH, P, 8), np.float32)
    xr = X.reshape(NCH, P, D)
    xm[:, :, 0:4] = xr[:, :, PM_MI]
    xm[:, :, 4:8] = xr[:, :, PM_MO]
    xmov = np.ascontiguousarray(
        xm.transpose(1, 0, 2).reshape(P, NCH * 8).astype(np.float16))

    in_maps = []
    for k in range(N_CORES):
        nk = slice(k * P, (k + 1) * P)
        sm = np.empty((P, SM_W), np.float32)
        sm[:, SM_XK:SM_XK + 4] = X[nk][:, XK_PERM]
        sm[:, SM_CK:] = ck1[None, :]
        in_maps.append({
            "amat": _pack_amat(np.ascontiguousarray(A[nk, :].T),
                               np.ascontiguousarray(A[:, nk])),
            "xmov": xmov,
            "smalls": np.ascontiguousarray(sm),
        })

    nc = _get_nc()
    res = run_bass_kernel_spmd(nc, in_maps, core_ids=list(range(N_CORES)),
                               **_RUN_KWARGS)
    _LAST_RESULTS.clear()
    _LAST_RESULTS.append(res)
    return np.concatenate(
        [res.results[k]["out"].reshape(-1) for k in range(N_CORES)]
    ).astype(np.float32)


# revision 41
# speedup vs baseline: 1.1911x; 1.1466x over previous
"""Trainium2 Bass kernel for nn_NodeNet: GNN message passing + 12-qubit TTN circuit.

Math: the reference's statevector circuit contracts exactly to per-node
Bloch-vector chains (every CNOT block keeps only its target wire; the
measurement is <Z_9>; the circuit is a tree so alive wires stay in
product states). Per node the whole circuit is ~60 scalar ops.

Message passing: Ri/Ro are one-hot column selection matrices, so
  mi = (Ri*e) @ Ro^T @ X = A @ X,   mo = (Ro*e) @ Ri^T @ X = A^T @ X
with A[n,m] = sum_{e: idx_i[e]=n, idx_o[e]=m} e[e] a [1024,1024] graph
matrix built on the host from the weights alone (like the theta->SO(3)
prep). Sharding is then data-parallel over nodes with NO collective:
core k loads A[nk,:]^T and A[:,nk] column-panels (bf16 high+low split,
fp32-grade) and contracts them against the replicated X (also split,
feature-permuted per destination angle slot) in 32 tiny matmuls.

Per-core HBM traffic is ~1.1 MB vs 8 MB for the dense-relation
formulation, and the ReduceScatter (40us wall) is gone entirely.
"""

import ml_dtypes
import numpy as np

import bass_rust
import concourse.bass as bass
import concourse.mybir as mybir
import concourse.tile as tile
from concourse.bass_utils import run_bass_kernel_spmd
from concourse.masks import make_identity

F32 = mybir.dt.float32
F16 = mybir.dt.float16
N_CORES = 8
N, E, D = 1024, 8192, 4
P = 128                  # partitions / nodes per core
NCH = N // P             # 8 global-node chunks (contraction dim)

_BLOCKS = [(0, 1, (0, 1)), (2, 3, (3, 2)), (4, 5, (4, 5)), (6, 7, (7, 6)),
           (8, 9, (8, 9)), (10, 11, (11, 10)), (1, 2, (1, 2)), (5, 6, (6, 5)),
           (9, 10, (10, 9)), (2, 5, (2, 5)), (5, 9, (5, 9))]

# ---------------------------------------------------------------------------
# Column layout of the M-angle tile
# ---------------------------------------------------------------------------
# M cols 0:6  = layer-A target wires  [w1, w6, w10, w2, w5, w9]
# M cols 6:12 = layer-A control wires [w0, w7, w11, w3, w4, w8] (block-paired)
# Sources: wire w<4 -> mi[:,w]; 4<=w<8 -> mo[:,w-4]; w>=8 -> X[:,w-8]
#   mi lands at cols {0,3,6,9} (stride 3): order [mi1, mi2, mi0, mi3]
#   mo lands at cols {1,4,7,10}: order [mo2, mo1, mo3, mo0]
#   X  lands at cols {2,5,8,11}: order [X2, X1, X3, X0]
A_BLOCKS = [0, 3, 5, 1, 2, 4]     # block idx per A-target col
B_BLOCKS = [6, 7, 8]              # b-cols [w2, w5, w9] <- a-cols [w1, w6, w10]
PM_MI = [1, 2, 0, 3]              # mi feature order in M stride-3 slots
PM_MO = [2, 1, 3, 0]              # mo feature order
XK_PERM = [2, 1, 3, 0]            # X columns in M stride-3 order

# ---------------------------------------------------------------------------
# Host-side circuit-constant preparation
# ---------------------------------------------------------------------------

_PAULI = np.array([
    [[0, 1], [1, 0]],
    [[0, -1j], [1j, 0]],
    [[1, 0], [0, -1]],
], dtype=np.complex128)


def _rot_so3(p):
    """SO(3) Bloch rotation of Rot(phi, theta, omega) = RZ(om) RY(th) RZ(phi)."""
    phi, th, om = float(p[0]), float(p[1]), float(p[2])
    c, s = np.cos(th / 2), np.sin(th / 2)
    U = np.array([
        [np.exp(-0.5j * (phi + om)) * c, -np.exp(0.5j * (phi - om)) * s],
        [np.exp(-0.5j * (phi - om)) * s, np.exp(0.5j * (phi + om)) * c],
    ])
    R = np.empty((3, 3))
    for i in range(3):
        for j in range(3):
            R[i, j] = 0.5 * np.real(
                np.trace(_PAULI[i] @ U @ _PAULI[j] @ U.conj().T))
    return R


# circuit-constants column layout (offsets into the ck segment of smalls)
# Layer A groups are 12 wide, col 2t+h (t = A-target col, h = 0:x-term
# j2=0 / 1:z-term j2=2) so one TT against the (q, h) sin view + a
# tensor_reduce over h computes all 6 blocks of one component.
# Layer B groups are 9 wide, col 3s+g (s = B block, g = xyz component).
CK_AZ6 = 0       # layer A control row2: Rc_t[2, j2(h)]
CK_AB = 12       # layer A target: 3 groups of 12, Rt_t[i, j2(h)]
CK_AZ3 = 48      # layer B control row2: Rc_s[2, g]
CK_BB = 57       # layer B target: 3 groups of 9, Rt_s[i, g]
CK_C19 = 84      # R19 column-major: col 3c+i = R19[i, c]
CK_C18 = 93      # R18 row 2 (block 9 control rot)
CK_C21 = 96      # R21 row 2 (block 10 target rot)
CK_C20 = 99      # R20 row 2 (block 10 control rot)
CK_W = 102

# smalls tensor layout: [xk_perm(4) | ck(CK_W)]
SM_XK = 0
SM_CK = 4
SM_W = SM_CK + CK_W


def _pack_ck(theta):
    th = np.asarray(theta, np.float64)
    R = [_rot_so3(th[3 * k:3 * k + 3]) for k in range(23)]
    ck = np.zeros(CK_W, np.float64)

    for t, bidx in enumerate(A_BLOCKS):
        w1, w2, (c, tt) = _BLOCKS[bidx]
        k1, k2 = 2 * bidx, 2 * bidx + 1
        Rc = R[k1] if c == w1 else R[k2]
        Rt = R[k1] if tt == w1 else R[k2]
        for h, j2 in enumerate((0, 2)):
            ck[CK_AZ6 + 2 * t + h] = Rc[2, j2]
            for i in range(3):
                ck[CK_AB + 12 * i + 2 * t + h] = Rt[i, j2]

    for s, bidx in enumerate(B_BLOCKS):
        w1, w2, (c, tt) = _BLOCKS[bidx]
        k1, k2 = 2 * bidx, 2 * bidx + 1
        Rc = R[k1] if c == w1 else R[k2]
        Rt = R[k1] if tt == w1 else R[k2]
        for g in range(3):
            ck[CK_AZ3 + 3 * s + g] = Rc[2, g]
            for i in range(3):
                ck[CK_BB + 9 * i + 3 * s + g] = Rt[i, g]

    # layer C: block 9 = (2,5,(2,5)): control rot R[18] (wire2), target R[19]
    #          block 10 = (5,9,(5,9)): control rot R[20] (wire5), target R[21]
    ck[CK_C19:CK_C19 + 9] = R[19].T.reshape(-1)
    ck[CK_C18:CK_C18 + 3] = R[18][2]
    ck[CK_C21:CK_C21 + 3] = -np.pi * R[21][2]   # folds res = -pi*zf + pi
    ck[CK_C20:CK_C20 + 3] = R[20][2]
    return ck.astype(np.float32)


# ---------------------------------------------------------------------------
# Walrus workaround: this build rejects >1 sync-wait per instruction
# ---------------------------------------------------------------------------


def _split_multi_waits(nc):
    for f in nc.m.functions:
        for bb in f.blocks:
            out = []
            for inst in bb.instructions:
                si = inst.sync_info
                if si is not None and si.on_wait and len(si.on_wait) > 1:
                    waits = list(si.on_wait)
                    for i, w in enumerate(waits[:-1]):
                        out.append(mybir.InstNoOp(
                            name=f"{inst.name}_wsplit{i}",
                            engine=inst.engine,
                            ins=[], outs=[],
                            sync_info=bass_rust.SyncInfo(
                                on_wait=[w], on_update=[]),
                        ))
                    inst.sync_info = bass_rust.SyncInfo(
                        on_wait=[waits[-1]], on_update=list(si.on_update))
                out.append(inst)
            bb.instructions = out


# ---------------------------------------------------------------------------
# Device kernel
# ---------------------------------------------------------------------------


def _build_nc():
    nc = bass.Bass("TRN2", target_bir_lowering=False, num_devices=N_CORES)

    # A panels fp16 (5e-4 A rel err -> <1e-4 on the output, 200x inside
    # tolerance), both rels interleaved by half so each DMA piece moves
    # 2 KB per partition row (the efficient DMA line size):
    # cols [0:512 i-chunks 0-3 | 512:1024 o-chunks 0-3 | i 4-7 | o 4-7]
    amat = nc.declare_dram_parameter("amat", [P, 2 * NCH * P], F16,
                                     isOutput=False)
    # X moving: chunk c at cols 8c:8c+8 = [X permMI | X permMO]
    xmov_d = nc.declare_dram_parameter("xmov", [P, NCH * 8], F16,
                                       isOutput=False)
    smalls = nc.declare_dram_parameter("smalls", [P, SM_W], F32,
                                       isOutput=False)
    # single-partition row: a [128,1] output would DMA 128 4-byte
    # partition reads (~7us of descriptor overhead); [1,128] is one burst
    out = nc.declare_dram_parameter("out", [1, P], F32, isOutput=True)

    HPI = float(np.pi / 2)
    PI = float(np.pi)
    MUL = mybir.AluOpType.mult
    ADD = mybir.AluOpType.add

    with tile.TileContext(nc) as tc:
        with (
            tc.tile_pool(name="big", bufs=1) as big,
            tc.tile_pool(name="small", bufs=1) as small,
            tc.tile_pool(name="acc", bufs=2, space="PSUM") as accp,
            tc.tile_pool(name="tbp", bufs=1, space="PSUM") as tbp,
        ):
            # ---- DMAs: two [P, 1024] pieces, 2 KB per partition row ------
            HW = NCH * P // 2      # cols per half-panel
            pieces = [big.tile([P, 2 * HW], F16, name=f"piece{h}",
                               tag=f"piece{h}") for h in range(2)]
            nc.sync.dma_start(pieces[0][:], amat[:, 0:2 * HW])
            xm_sb = small.tile([P, NCH * 8], F16, name="xm_sb")
            nc.scalar.dma_start(xm_sb[:], xmov_d[:])
            nc.gpsimd.dma_start(pieces[1][:], amat[:, 2 * HW:4 * HW])
            sm_sb = small.tile([P, SM_W], F32, name="sm_sb")
            nc.scalar.dma_start(sm_sb[:], smalls[:])

            def ckc(off, n=1):
                return sm_sb[:, SM_CK + off:SM_CK + off + n]

            # preload the ACT Sin table set while DMAs stream
            warm = small.tile([P, 1], F32, name="warm")
            nc.vector.memset(warm[:], 0.0)
            nc.scalar.activation(warm[:], warm[:],
                                 mybir.ActivationFunctionType.Sin)
            pi_t = small.tile([P, 1], F32, name="pi_t")
            nc.vector.memset(pi_t[:], float(np.pi))
            ident = small.tile([P, P], F32, name="ident")
            make_identity(nc, ident)

            # ---- matmuls: mi = A[nk,:] X, mo = A[:,nk]^T X ---------------
            ps = {}
            for ri, rel in enumerate(("i", "o")):
                ps[rel] = accp.tile([P, 4], F32, name=f"ps_{rel}",
                                    tag=f"ps_{rel}")
            for h in range(2):            # half-panel: chunks 4h..4h+3
                for rel in ("i", "o"):
                    mo_off = 0 if rel == "i" else 4
                    rel_off = 0 if rel == "i" else HW
                    for cc in range(NCH // 2):
                        c = 4 * h + cc
                        nc.tensor.matmul(
                            ps[rel][:],
                            pieces[h][:, rel_off + cc * P:
                                      rel_off + cc * P + P],
                            xm_sb[:, c * 8 + mo_off:c * 8 + mo_off + 4],
                            start=(h == 0 and cc == 0),
                            stop=(h == 1 and cc == 3))

            # ---- circuit: build M angles ---------------------------------
            # cols 0:12 = m (stride-3 interleave), cols 12:24 = m + pi/2
            m_ang = small.tile([P, 24], F32, name="m_ang")
            m3 = m_ang.rearrange("p (c t) -> p c t", t=3)
            nc.vector.tensor_copy(m3[:, 0:4, 0], ps["i"][:])
            nc.vector.tensor_copy(m3[:, 0:4, 1], ps["o"][:])
            nc.gpsimd.tensor_copy(m3[:, 0:4, 2], sm_sb[:, SM_XK:SM_XK + 4])
            nc.vector.tensor_scalar(
                m_ang[:, 12:24], m_ang[:, 0:12], HPI, None, ADD)

            # range-reduce into [-pi, pi] via the magic-constant RNE trick:
            # t = rne(m/2pi) = (m/2pi + 1.5*2^23) - 1.5*2^23; m2 = m - 2pi*t
            TWO_PI = float(2 * np.pi)
            MAGIC = float(1.5 * 2 ** 23)
            tq = small.tile([P, 24], F32, name="tq")
            m2 = small.tile([P, 24], F32, name="m2")
            nc.vector.tensor_scalar(
                tq[:], m_ang[:], float(1.0 / TWO_PI), MAGIC, MUL, ADD)
            nc.vector.tensor_scalar(tq[:], tq[:], -MAGIC, None, ADD)
            nc.vector.scalar_tensor_tensor(
                m2[:], tq[:], -TWO_PI, m_ang[:], MUL, ADD)
            nc.vector.tensor_scalar(
                m2[:], m2[:], PI, -PI,
                mybir.AluOpType.min, mybir.AluOpType.max)
            sxz = small.tile([P, 24], F32, name="sxz")
            nc.scalar.activation(sxz[:], m2[:],
                                 mybir.ActivationFunctionType.Sin)

            TT = nc.vector.tensor_tensor
            TS = nc.vector.tensor_scalar
            STT = nc.vector.scalar_tensor_tensor
            RED = nc.vector.tensor_reduce
            AX = mybir.AxisListType.X

            # sin view indexed (q, h): col = 12h + q; q 0:6 targets (sxb,
            # szb), q 6:12 controls (sxa, sza)
            s_qh = sxz.rearrange("p (h q) -> p q h", h=2)

            # ---- layer A: one wide TT via stride-0 component broadcast ---
            # ab_cat = [abx(6) | aby(6) | abz(6)], az6 separate
            ab_cat = small.tile([P, 18], F32, name="ab_cat")
            az6 = small.tile([P, 6], F32, name="az6")
            t36 = small.tile([P, 36], F32, name="t36")
            t12 = small.tile([P, 12], F32, name="t12")
            tgt_b = s_qh[:, 0:6, :].unsqueeze(1).broadcast_to([P, 3, 6, 2])
            TT(t36[:], ckc(CK_AB, 36), tgt_b, MUL)
            RED(ab_cat[:], t36.rearrange("p (a h) -> p a h", h=2), AX, ADD)
            TT(t12[:], ckc(CK_AZ6, 12), s_qh[:, 6:12, :], MUL)
            RED(az6[:], t12.rearrange("p (q h) -> p q h", h=2), AX, ADD)
            az6b = az6[:].unsqueeze(1).broadcast_to([P, 2, 6])
            ab23 = ab_cat[:, 6:18].rearrange("p (g c) -> p g c", g=2)
            TT(ab23, ab23, az6b, MUL)

            # ---- layer B: same shape of fusion over (i, s, g) ------------
            # ab_cat viewed (s, g): col = 6g + s; s 0:3 = a-cols, 3:6 b-cols
            ab_sg = ab_cat.rearrange("p (g s) -> p s g", g=3)
            bb_cat = small.tile([P, 9], F32, name="bb_cat")
            az3 = small.tile([P, 3], F32, name="az3")
            t27 = small.tile([P, 27], F32, name="t27")
            t9 = small.tile([P, 9], F32, name="t9")
            bv_b = ab_sg[:, 3:6, :].unsqueeze(1).broadcast_to([P, 3, 3, 3])
            TT(t27[:], ckc(CK_BB, 27), bv_b, MUL)
            RED(bb_cat[:], t27.rearrange("p (i g) -> p i g", g=3), AX, ADD)
            TT(t9[:], ckc(CK_AZ3, 9), ab_sg[:, 0:3, :], MUL)
            RED(az3[:], t9.rearrange("p (s g) -> p s g", g=3), AX, ADD)
            az3b = az3[:].unsqueeze(1).broadcast_to([P, 2, 3])
            bb23 = bb_cat[:, 3:9].rearrange("p (g c) -> p g c", g=2)
            TT(bb23, bb23, az3b, MUL)

            # ---- layer C: blocks 9 then 10 -------------------------------
            # bb_cat cols: comp c of wire w at 3c + w' (w' 0=w2, 1=w5, 2=w9)
            # STT accum_out fuses each 3-term dot into one instruction;
            # CK_C21 is pre-scaled by -pi on the host so the final result
            # is a single fused multiply-add against the pi constant.
            s9 = small.tile([P, 1], F32, name="s9")
            u = small.tile([P, 1], F32, name="u")
            t3 = small.tile([P, 3], F32, name="t3")
            STT(t3[:], ckc(CK_C18, 3), 1.0, bb_cat[:, 0:9:3], MUL, MUL,
                accum_out=s9[:])
            STT(t3[:], ckc(CK_C21, 3), 1.0, bb_cat[:, 2:9:3], MUL, MUL,
                accum_out=u[:])

            # w5 rows: one broadcast TT + transposed-write reduce over c
            w5c = small.tile([P, 3], F32, name="w5c")
            t9c = small.tile([P, 9], F32, name="t9c")
            bb13b = bb_cat[:, 1:9:3].unsqueeze(2).broadcast_to([P, 3, 3])
            TT(t9c.rearrange("p (i c) -> p c i", i=3),
               ckc(CK_C19, 9), bb13b, MUL)
            RED(w5c[:], t9c.rearrange("p (i c) -> p i c", i=3), AX, ADD)
            TS(w5c[:, 1:3], w5c[:, 1:3], s9[:, 0:1], None, MUL)

            s10 = small.tile([P, 1], F32, name="s10")
            STT(t3[:], ckc(CK_C20, 3), 1.0, w5c[:], MUL, MUL,
                accum_out=s10[:])

            # res = s10 * (-pi*u) + pi
            res = small.tile([P, 1], F32, name="res")
            STT(res[:], s10[:], u[:, 0:1], pi_t[:], MUL, ADD)
            row_ps = tbp.tile([1, P], F32, name="row_ps", tag="row")
            nc.tensor.transpose(row_ps[:], res[:], ident[:])
            row_sb = small.tile([1, P], F32, name="row_sb")
            nc.vector.tensor_copy(row_sb[:], row_ps[:])
            nc.scalar.dma_start(out[:], row_sb[:])

    return nc


_NC_CACHE = {}
_RUN_KWARGS = {}      # test harness can set e.g. {"trace": True}
_LAST_RESULTS = []    # BassKernelResults of the most recent run


def _get_nc():
    if "nc" not in _NC_CACHE:
        nc = _build_nc()
        _split_multi_waits(nc)
        _NC_CACHE["nc"] = nc
    return _NC_CACHE["nc"]


def _build_graph_matrix(e, Ri, Ro):
    """A[n,m] = sum over edges (idx_i=n, idx_o=m) of e, in float64."""
    e64 = np.asarray(e, np.float64)
    Ri32 = np.asarray(Ri, np.float32)
    Ro32 = np.asarray(Ro, np.float32)
    idx_i = np.argmax(Ri32, axis=0)
    idx_o = np.argmax(Ro32, axis=0)
    if (np.count_nonzero(Ri32) == E and np.count_nonzero(Ro32) == E
            and np.all(Ri32[idx_i, np.arange(E)] == 1.0)
            and np.all(Ro32[idx_o, np.arange(E)] == 1.0)):
        A = np.zeros((N, N), np.float64)
        np.add.at(A, (idx_i, idx_o), e64)
        return A
    # general fallback (never hit for one-hot relation inputs)
    return (Ri32.astype(np.float64) * e64) @ Ro32.astype(np.float64).T


def _pack_amat(Mi64, Mo64):
    """Two [1024, 128] panels -> [128, 2048] fp16: piece h holds chunks
    4h..4h+3 of rel i then rel o; within a rel, chunk c at cols 128c."""
    ri = Mi64.astype(np.float16).reshape(2, 4, P, P)   # [h, cc, p, l]
    ro = Mo64.astype(np.float16).reshape(2, 4, P, P)
    both = np.stack([ri, ro], axis=1)                  # [h, rel, cc, p, l]
    return np.ascontiguousarray(
        both.transpose(3, 0, 1, 2, 4).reshape(P, 2 * NCH * P))


def kernel(X, e, Ri, Ro, theta):
    X = np.ascontiguousarray(np.asarray(X, np.float32))
    e = np.ascontiguousarray(np.asarray(e, np.float32))
    theta = np.asarray(theta, np.float32)

    A = _build_graph_matrix(e, Ri, Ro)
    ck1 = _pack_ck(theta)

    # X moving operand: fp16, feature-permuted per destination angle slot
    xm = np.zeros((NCH, P, 8), np.float32)
    xr = X.reshape(NCH, P, D)
    xm[:, :, 0:4] = xr[:, :, PM_MI]
    xm[:, :, 4:8] = xr[:, :, PM_MO]
    xmov = np.ascontiguousarray(
        xm.transpose(1, 0, 2).reshape(P, NCH * 8).astype(np.float16))

    in_maps = []
    for k in range(N_CORES):
        nk = slice(k * P, (k + 1) * P)
        sm = np.empty((P, SM_W), np.float32)
        sm[:, SM_XK:SM_XK + 4] = X[nk][:, XK_PERM]
        sm[:, SM_CK:] = ck1[None, :]
        in_maps.append({
            "amat": _pack_amat(np.ascontiguousarray(A[nk, :].T),
                               np.ascontiguousarray(A[:, nk])),
            "xmov": xmov,
            "smalls": np.ascontiguousarray(sm),
        })

    nc = _get_nc()
    res = run_bass_kernel_spmd(nc, in_maps, core_ids=list(range(N_CORES)),
                               **_RUN_KWARGS)
    _LAST_RESULTS.clear()
    _LAST_RESULTS.append(res)
    return np.concatenate(
        [res.results[k]["out"].reshape(-1) for k in range(N_CORES)]
    ).astype(np.float32)
